# revision 14
# baseline (speedup 1.0000x reference)
"""Trainium2 Bass kernel for nn_BIMM1D (Gaussian-mixture NLL loss).

Math: loss = -(1/M) sum_m log p(u_m), where p(u) is a 772-atom Gaussian
mixture (4 interior + 6x128 MC interface atoms, shared sigma_n) that is the
SAME 1-D function of u for every data point.

Strategy (per core, data-parallel over 8 cores; one packed input DMA):
  Stream A (ACT+PE): evaluate S(x) = sum_j w_j exp(-((x-c_j)/(sqrt2 sn))^2)
    at G=128 Chebyshev nodes (7 Derivative_Erf passes, one per atom group,
    softmax weights normalized on device and folded into the PE reduction
    lhsT=E_g, rhs=w_g -> S accumulates as a PSUM column), take Ln (-> SBUF
    column), and fit a degree-13 polynomial in t = affine(x) with one
    matmul against a constant pseudo-inverse matrix (pure layout constant).
    All data-dependent math is on device (erf for MC centers, softmax via
    the sigmoid identity e^x = s/(1-s), the table, Ln, the fit).
  Stream B (DVE+Pool+PE): map the 32768-point u shard [128,256] to t,
    build monomial powers t^2..t^13 (tensor_tensor mults; high powers on
    GPSIMD), and reduce each power to per-partition column sums with PE
    matmuls (lhsT=power half, rhs=ones) accumulated into pcols[:,k].
  Converge: phi = ones^T-reduction of pcols, sum_m logS(u_m) ~= c . phi
    (one [14]x[14] PE dot); host adds the closed-form constant C0(sn) and
    sums the 8 per-core partials.

Accuracy: the degree-13 fit has ~3e-3 sup error on [0,1] but the empirical
mean over 262144 ~uniform points concentrates (measured end-to-end f32 rel
err ~1e-4 against the f64 reference, vs 2e-2 tolerance).
"""
import os
import sys
import math
import numpy as np

for _p in ("/opt/trn_rl_repo", "/root/.axon_site/_ro/trn_rl_repo"):
    if os.path.isdir(_p) and _p not in sys.path:
        sys.path.insert(0, _p)

import concourse.bass as bass
import concourse.bacc as bacc
import concourse.mybir as mybir
import concourse.tile as tile
from concourse.bass_utils import run_bass_kernel_spmd
from contextlib import ExitStack

dt = mybir.dt
AF = mybir.ActivationFunctionType
ALU = mybir.AluOpType

# ---- static problem geometry (hardcoded per contract) ----
M_TOTAL = 262144
N_CORES = 8
M_SHARD = M_TOTAL // N_CORES          # 32768
SW = M_SHARD // 128                   # 256 columns in wrapped layout
N_MC = 128                            # MC samples per interface
N_PAIRS = 6
N_PHASES = 4
N_GROUPS = 7                          # 6 interface groups + 1 interior group
NW = N_PHASES + N_PAIRS               # 10 mixture weights
SQRT2 = math.sqrt(2.0)

# ---- fit geometry ----
G = 64                                # Chebyshev fit nodes
DEG = 13                              # polynomial degree
NC_ = DEG + 1                         # 14 coefficients
LO, HI = -0.02, 1.02                  # fit interval (u in [0,1))
MID = 0.5 * (LO + HI)
INV = 2.0 / (HI - LO)

_IA = [0, 0, 0, 1, 1, 2]
_IB = [1, 2, 3, 2, 3, 3]

# packed input layout: [128, NPK] f32
#   col 0: sn (replicated), col 1: d (replicated), col 2: I4 (zero-padded)
#   row 0 cols 4:14: W; rows 0:6 cols 16:144: eps; cols 144:400: u wrapped
C_SN, C_D, C_I4, C_W, C_EPS, C_U = 0, 1, 2, 4, 16, 144
NPK = C_U + SW                        # 400

# power factorization t^k = t^(k//2) * t^(k-k//2), k = 2..DEG
_POW_FACT = [(k // 2, k - k // 2) for k in range(2, DEG + 1)]
POOL_MIN_POW = 9                      # powers >= this run on GPSIMD

_cache = {}
last_exec_time_ns = None
last_results = None


def _build_nc(repeat=1, debug_outs=False):
    nc = bacc.Bacc("TRN2", target_bir_lowering=False, debug=False)
    f32 = dt.float32

    packed_d = nc.dram_tensor("packed", [128, NPK], f32, kind="ExternalInput")
    # layout constants
    nodes_d = nc.dram_tensor("nodes", [G], f32, kind="ExternalInput")
    pinvT_d = nc.dram_tensor("pinvT", [G, NC_], f32, kind="ExternalInput")
    sela_d = nc.dram_tensor("sela", [N_PHASES, N_PAIRS], f32, kind="ExternalInput")
    selb_d = nc.dram_tensor("selb", [N_PHASES, N_PAIRS], f32, kind="ExternalInput")
    id6_d = nc.dram_tensor("ident6", [N_PAIRS, N_PAIRS], f32, kind="ExternalInput")
    onesr_d = nc.dram_tensor("ones_row", [1, 128], f32, kind="ExternalInput")
    onesc_d = nc.dram_tensor("ones_col", [128, 1], f32, kind="ExternalInput")
    out_d = nc.dram_tensor("out", [1, 1], f32, kind="ExternalOutput")
    if debug_outs:
        dbgc_d = nc.dram_tensor("dbg_c", [NC_, 1], f32, kind="ExternalOutput")
        dbgf_d = nc.dram_tensor("dbg_fcol", [G, 1], f32, kind="ExternalOutput")
        dbgp_d = nc.dram_tensor("dbg_phi", [NC_, 1], f32, kind="ExternalOutput")

    with tile.TileContext(nc) as tc, ExitStack() as ctx:
        cpool = ctx.enter_context(tc.tile_pool(name="consts", bufs=1))
        wpool = ctx.enter_context(tc.tile_pool(name="work", bufs=1))
        gpool = ctx.enter_context(tc.tile_pool(name="gwork", bufs=3))
        pps = ctx.enter_context(tc.tile_pool(name="pps", bufs=1, space="PSUM"))

        # ---- constants loaded once ----
        onesr_t = cpool.tile([1, 128], f32, tag="onesr")
        nc.sync.dma_start(onesr_t[:], onesr_d.ap())
        onesc_t = cpool.tile([128, 1], f32, tag="onesc")
        nc.sync.dma_start(onesc_t[:], onesc_d.ap())
        sela_t = cpool.tile([N_PHASES, N_PAIRS], f32, tag="sela")
        nc.sync.dma_start(sela_t[:], sela_d.ap())
        selb_t = cpool.tile([N_PHASES, N_PAIRS], f32, tag="selb")
        nc.sync.dma_start(selb_t[:], selb_d.ap())
        id6_t = cpool.tile([N_PAIRS, N_PAIRS], f32, tag="id6")
        nc.sync.dma_start(id6_t[:], id6_d.ap())
        pinvT_t = cpool.tile([G, NC_], f32, tag="pinvT")
        nc.sync.dma_start(pinvT_t[:], pinvT_d.ap())
        # node coordinates replicated to all 128 partitions: [128, G]
        xrep_t = cpool.tile([128, G], f32, tag="xrep")
        nc.sync.dma_start(
            xrep_t[:],
            nodes_d.ap().rearrange("(a b) -> a b", a=1).to_broadcast((128, G)),
        )

        def body():
            # ---- one packed input DMA ----
            pk_t = wpool.tile([128, NPK], f32, tag="packed")
            nc.sync.dma_start(pk_t[:], packed_d.ap())
            sncol = pk_t[:, C_SN:C_SN + 1]
            dcol6 = pk_t[0:N_PAIRS, C_D:C_D + 1]
            i4col = pk_t[:, C_I4:C_I4 + 1]
            i4_t = pk_t[0:N_PHASES, C_I4:C_I4 + 1]
            wrow = pk_t[0:1, C_W:C_W + NW]
            eps_t = pk_t[0:N_PAIRS, C_EPS:C_EPS + N_MC]
            usb = pk_t[:, C_U:C_U + SW]

            # PSUM scratch (column-sliced; 4 banks total)
            ptiny = pps.tile([128, 8], f32, tag="ptiny")
            pwide = pps.tile([128, 16], f32, tag="pwide")
            pcols = pps.tile([128, NC_], f32, tag="pcols")

            # ====== latency-critical prep that gates the ACT stream ======
            with tc.high_priority():
                # dummy Erf: forces the sigmoid-set table load to start at
                # iteration t=0 (no data deps) instead of after e1's inputs
                pre = wpool.tile([1, 1], f32, tag="pre")
                nc.scalar.activation(pre[:], onesr_t[0:1, 0:1], AF.Erf)
                # softmax prep -> sigmoid
                m11 = wpool.tile([1, 1], f32, tag="m11")
                nc.vector.reduce_max(m11[:], wrow, axis=mybir.AxisListType.X)
                wm = wpool.tile([1, NW], f32, tag="wm")
                nc.vector.tensor_scalar(wm[:], wrow, m11[:], None, ALU.subtract)
                # erf scale/bias from the replicated d column
                scale_erf = wpool.tile([N_PAIRS, 1], f32, tag="scale_erf")
                nc.vector.tensor_scalar_mul(scale_erf[:], dcol6, SQRT2)
                bias_erf = wpool.tile([N_PAIRS, 1], f32, tag="bias_erf")
                nc.vector.tensor_scalar_mul(bias_erf[:], dcol6, -1.0 / SQRT2)
                # kcol = 1/(sqrt2*sn)
                iscol = wpool.tile([128, 1], f32, tag="iscol")
                nc.vector.reciprocal(iscol[:], sncol)
                kcol = wpool.tile([128, 1], f32, tag="kcol")
                nc.vector.tensor_scalar_mul(kcol[:], iscol[:], 1.0 / SQRT2)

                # ACT: erf for interface centers (loads sigmoid set), sigmoid
                e1 = wpool.tile([N_PAIRS, N_MC], f32, tag="e1")
                nc.scalar.activation(e1[:], eps_t, AF.Erf,
                                     bias=bias_erf[:], scale=scale_erf[:])
                sig = wpool.tile([1, NW], f32, tag="sig")
                nc.scalar.activation(sig[:], wm[:], AF.Sigmoid)

                # interface centers cin [6,128] -> ccT -> biasz
                iac = ptiny[0:N_PAIRS, 0:1]
                nc.tensor.matmul(iac, sela_t[:], i4_t, start=True, stop=True)
                ibc = ptiny[0:N_PAIRS, 1:2]
                nc.tensor.matmul(ibc, selb_t[:], i4_t, start=True, stop=True)
                iacol = wpool.tile([N_PAIRS, 1], f32, tag="iacol")
                nc.vector.tensor_copy(iacol[:], iac)
                hdiff = wpool.tile([N_PAIRS, 1], f32, tag="hdiff")
                nc.vector.tensor_tensor(hdiff[:], ibc, iacol[:], ALU.subtract)
                nc.vector.tensor_scalar_mul(hdiff[:], hdiff[:], 0.5)
                bsum = wpool.tile([N_PAIRS, 1], f32, tag="bsum")
                nc.vector.tensor_tensor(bsum[:], iacol[:], hdiff[:], ALU.add)
                cin = wpool.tile([N_PAIRS, N_MC], f32, tag="cin")
                nc.vector.tensor_scalar(cin[:], e1[:], hdiff[:], bsum[:],
                                        ALU.mult, ALU.add)
                ccT = pwide[:, 0:N_PAIRS]
                nc.tensor.transpose(ccT, cin[:], id6_t[:])
                biasz = wpool.tile([128, N_GROUPS], f32, tag="biasz")
                nc.vector.tensor_scalar(biasz[:, 0:N_PAIRS], ccT, kcol[:],
                                        -1.0, ALU.mult, ALU.mult)
                nc.vector.tensor_scalar(biasz[:, N_PAIRS:N_GROUPS], i4col,
                                        kcol[:], -1.0, ALU.mult, ALU.mult)

                # normalized weights: eec = (sig/(1-sig)) / se  (= softmax)
                omse = wpool.tile([1, NW], f32, tag="omse")
                nc.vector.tensor_scalar(omse[:], sig[:], -1.0, 1.0,
                                        ALU.mult, ALU.add)
                rec = wpool.tile([1, NW], f32, tag="rec")
                nc.vector.reciprocal(rec[:], omse[:])
                ee = wpool.tile([1, NW], f32, tag="ee")
                nc.vector.tensor_tensor(ee[:], sig[:], rec[:], ALU.mult)
                se = wpool.tile([1, 1], f32, tag="se")
                nc.vector.reduce_sum(se[:], ee[:], axis=mybir.AxisListType.X)
                rse = wpool.tile([1, 1], f32, tag="rse")
                nc.vector.reciprocal(rse[:], se[:])
                eec = wpool.tile([1, NW], f32, tag="eec")
                nc.vector.tensor_scalar(eec[:], ee[:], rse[:], None, ALU.mult)

                # weight columns [128, 7]
                p_eeT = ptiny[0:NW, 2:3]
                nc.tensor.matmul(p_eeT, eec[:], onesr_t[0:1, 0:1],
                                 start=True, stop=True)
                p_wif = pwide[:, 8:8 + N_PAIRS]
                nc.tensor.matmul(p_wif, onesr_t[:], eec[0:1, N_PHASES:],
                                 start=True, stop=True)
                wcols = wpool.tile([128, N_GROUPS], f32, tag="wcols")
                nc.vector.tensor_scalar_mul(wcols[:, 0:N_PAIRS], p_wif,
                                            1.0 / N_MC)
                nc.vector.memset(wcols[:, N_PAIRS:N_GROUPS], 0.0)
                nc.vector.tensor_copy(wcols[0:N_PHASES, N_PAIRS:N_GROUPS],
                                      p_eeT[0:N_PHASES, :])

            # ================= stream A: table + fit =================
            pScol = ptiny[0:G, 3:4]
            for g in range(N_GROUPS):
                eg = gpool.tile([128, G], f32, tag="eg")
                nc.scalar.activation(eg[:], xrep_t[:], AF.Derivative_Erf,
                                     bias=biasz[:, g:g + 1], scale=kcol[:])
                nc.tensor.matmul(pScol, eg[:], wcols[:, g:g + 1],
                                 start=(g == 0), stop=(g == N_GROUPS - 1))
            fcol = wpool.tile([G, 1], f32, tag="fcol")
            nc.scalar.activation(fcol[:], pScol, AF.Ln)
            c_p = ptiny[0:NC_, 4:5]
            nc.tensor.matmul(c_p, pinvT_t[:], fcol[:], start=True, stop=True)
            c_sb = wpool.tile([NC_, 1], f32, tag="c_sb")
            nc.vector.tensor_copy(c_sb[:], c_p)

            # ================= stream B: moments =================
            pows = wpool.tile([128, NC_ * SW], f32, tag="pows")  # slot d: t^d

            def slot(d):
                return pows[:, d * SW:(d + 1) * SW]

            HF = SW // 2

            def msum(k):
                nc.tensor.matmul(pcols[:, k:k + 1], slot(k)[:, 0:HF],
                                 onesc_t[:], start=True, stop=False)
                nc.tensor.matmul(pcols[:, k:k + 1], slot(k)[:, HF:SW],
                                 onesc_t[:], start=False, stop=True)

            tmap = wpool.tile([128, SW], f32, tag="tmap")
            nc.vector.tensor_scalar(tmap[:], usb, MID, INV,
                                    ALU.subtract, ALU.mult)
            nc.vector.tensor_scalar(slot(1), tmap[:], -1.0, 1.0,
                                    ALU.max, ALU.min)
            msum(1)
            for k in range(2, DEG + 1):
                a, b = _POW_FACT[k - 2]
                eng = nc.gpsimd if k >= POOL_MIN_POW else nc.vector
                eng.tensor_tensor(slot(k), slot(a), slot(b), ALU.mult)
                msum(k)
            pcols_sb = wpool.tile([128, NC_], f32, tag="pcols_sb")
            nc.vector.tensor_copy(pcols_sb[:, 1:NC_], pcols[:, 1:NC_])
            nc.vector.memset(pcols_sb[:, 0:1], float(SW))
            phi_p = ptiny[0:NC_, 5:6]
            nc.tensor.matmul(phi_p, pcols_sb[:], onesc_t[:],
                             start=True, stop=True)
            phi_sb = wpool.tile([NC_, 1], f32, tag="phi_sb")
            nc.vector.tensor_copy(phi_sb[:], phi_p)

            # ================= converge =================
            pout = pwide[0:1, 14:15]
            nc.tensor.matmul(pout, c_sb[:], phi_sb[:], start=True, stop=True)
            out_sb = wpool.tile([1, 1], f32, tag="out_sb")
            nc.vector.tensor_copy(out_sb[:], pout)
            nc.sync.dma_start(out_d.ap(), out_sb[:])
            if debug_outs:
                nc.sync.dma_start(dbgc_d.ap(), c_sb[:])
                nc.sync.dma_start(dbgf_d.ap(), fcol[:])
                nc.sync.dma_start(dbgp_d.ap(), phi_sb[:])

        if repeat == 1:
            body()
        else:
            with tc.For_i(0, repeat, 1):
                body()

    nc.compile()
    return nc


def _consts():
    ia = np.zeros((N_PHASES, N_PAIRS), np.float32)
    ib = np.zeros((N_PHASES, N_PAIRS), np.float32)
    for p, (a, b) in enumerate(zip(_IA, _IB)):
        ia[a, p] = 1.0
        ib[b, p] = 1.0
    # Chebyshev nodes on [LO, HI] and monomial-basis fit pseudo-inverse
    i = np.arange(G)
    tnodes = np.cos(np.pi * (2 * i + 1) / (2 * G))
    xnodes = (tnodes + 1) / 2 * (HI - LO) + LO
    V = np.vander(tnodes, NC_, increasing=True)      # [G, NC_] float64
    pinvT = np.linalg.pinv(V).T.astype(np.float32)   # [G, NC_]
    return {
        "nodes": xnodes.astype(np.float32),
        "pinvT": pinvT,
        "sela": ia,
        "selb": ib,
        "ident6": np.eye(N_PAIRS, dtype=np.float32),
        "ones_row": np.ones((1, 128), np.float32),
        "ones_col": np.ones((128, 1), np.float32),
    }


def make_in_maps(u, uniform_eps, I, sigma_n, d, W):
    u = np.asarray(u, np.float32).reshape(M_TOTAL)
    sn_v = np.float32(np.asarray(sigma_n).reshape(-1)[0])
    d_v = np.float32(np.asarray(d).reshape(-1)[0])
    base = np.zeros((128, NPK), np.float32)
    base[:, C_SN] = sn_v
    base[:, C_D] = d_v
    base[0:N_PHASES, C_I4] = np.asarray(I, np.float32).reshape(N_PHASES)
    base[0, C_W:C_W + NW] = np.asarray(W, np.float32).reshape(NW)
    base[0:N_PAIRS, C_EPS:C_EPS + N_MC] = np.asarray(
        uniform_eps, np.float32).reshape(N_PAIRS, N_MC)
    consts = _consts()
    in_maps = []
    for c in range(N_CORES):
        m = dict(consts)
        pk = base.copy()
        pk[:, C_U:C_U + SW] = u[c * M_SHARD:(c + 1) * M_SHARD].reshape(128, SW)
        m["packed"] = pk
        in_maps.append(m)
    return in_maps


def kernel(u, uniform_eps, I, sigma_b, sigma_n, d, W, n_MC_components=None):
    global last_exec_time_ns, last_results
    in_maps = make_in_maps(u, uniform_eps, I, sigma_n, d, W)

    key = "nc_dbg" if os.environ.get("KERNEL_DEBUG") else "nc"
    if key not in _cache:
        _cache[key] = _build_nc(debug_outs=bool(os.environ.get("KERNEL_DEBUG")))
    nc = _cache[key]

    trace = bool(int(os.environ.get("KERNEL_TRACE", "0")))
    res = run_bass_kernel_spmd(nc, in_maps, core_ids=list(range(N_CORES)),
                               trace=trace)
    last_results = res
    last_exec_time_ns = res.exec_time_ns

    total = sum(float(res.results[c]["out"][0, 0]) for c in range(N_CORES))
    sn_f = float(np.asarray(sigma_n).reshape(-1)[0])
    c0 = math.log(math.sqrt(math.pi) / 2.0) - math.log(math.sqrt(2.0 * math.pi) * sn_f)
    loss = -(total / M_TOTAL + c0)
    return np.float32(loss)


# revision 23
# speedup vs baseline: 1.4571x; 1.4571x over previous
"""Trainium2 Bass kernel for nn_BIMM1D (Gaussian-mixture NLL loss).

Math: loss = -(1/M) sum_m log p(u_m), where p(u) is a 772-atom Gaussian
mixture (4 interior + 6x128 MC interface atoms, shared sigma_n) that is the
SAME 1-D function of u for every data point.

Strategy (per core, data-parallel over 8 cores; one packed input DMA):
  Stream A (ACT+PE): evaluate S(x) = sum_j w_j exp(-((x-c_j)/(sqrt2 sn))^2)
    at G=128 Chebyshev nodes (7 Derivative_Erf passes, one per atom group,
    softmax weights normalized on device and folded into the PE reduction
    lhsT=E_g, rhs=w_g -> S accumulates as a PSUM column), take Ln (-> SBUF
    column), and fit a degree-13 polynomial in t = affine(x) with one
    matmul against a constant pseudo-inverse matrix (pure layout constant).
    All data-dependent math is on device (erf for MC centers, softmax via
    the sigmoid identity e^x = s/(1-s), the table, Ln, the fit).
  Stream B (DVE+Pool+PE): map the 32768-point u shard [128,256] to t,
    build monomial powers t^2..t^13 (tensor_tensor mults; high powers on
    GPSIMD), and reduce each power to per-partition column sums with PE
    matmuls (lhsT=power half, rhs=ones) accumulated into pcols[:,k].
  Converge: phi = ones^T-reduction of pcols, sum_m logS(u_m) ~= c . phi
    (one [14]x[14] PE dot); host adds the closed-form constant C0(sn) and
    sums the 8 per-core partials.

Accuracy: the degree-13 fit has ~3e-3 sup error on [0,1] but the empirical
mean over 262144 ~uniform points concentrates (measured end-to-end f32 rel
err ~1e-4 against the f64 reference, vs 2e-2 tolerance).
"""
import os
import sys
import math
import numpy as np

for _p in ("/opt/trn_rl_repo", "/root/.axon_site/_ro/trn_rl_repo"):
    if os.path.isdir(_p) and _p not in sys.path:
        sys.path.insert(0, _p)

import concourse.bass as bass
import concourse.bacc as bacc
import concourse.mybir as mybir
import concourse.tile as tile
from concourse.bass_utils import run_bass_kernel_spmd
from contextlib import ExitStack

dt = mybir.dt
AF = mybir.ActivationFunctionType
ALU = mybir.AluOpType

# ---- static problem geometry (hardcoded per contract) ----
M_TOTAL = 262144
N_CORES = 8
M_SHARD = M_TOTAL // N_CORES          # 32768
SW = M_SHARD // 128                   # 256 columns in wrapped layout
N_MC = 128                            # MC samples per interface
N_PAIRS = 6
N_PHASES = 4
N_GROUPS = 7                          # 6 interface groups + 1 interior group
NW = N_PHASES + N_PAIRS               # 10 mixture weights
SQRT2 = math.sqrt(2.0)

# ---- fit geometry ----
G = 64                                # Chebyshev fit nodes
DEG = 13                              # polynomial degree
NC_ = DEG + 1                         # 14 coefficients
LO, HI = -0.02, 1.02                  # fit interval (u in [0,1))
MID = 0.5 * (LO + HI)
INV = 2.0 / (HI - LO)

_IA = [0, 0, 0, 1, 1, 2]
_IB = [1, 2, 3, 2, 3, 3]

# packed input layout: [128, NPK] f32
#   col 0: sn (replicated), col 1: d (replicated), col 2: I4 (zero-padded)
#   row 0 cols 4:14: W; rows 0:6 cols 16:144: eps; cols 144:400: u wrapped
C_SN, C_D, C_I4, C_W, C_EPS, C_U = 0, 1, 2, 4, 16, 144
NPK = C_U + SW                        # 400

# power factorization t^k = t^(k//2) * t^(k-k//2), k = 2..DEG
_POW_FACT = [(k // 2, k - k // 2) for k in range(2, DEG + 1)]
POOL_MIN_POW = 9                      # powers >= this run on GPSIMD

_cache = {}
last_exec_time_ns = None
last_results = None


def _build_nc(repeat=1, debug_outs=False, ablate=()):
    ablate = set(ablate)
    nc = bacc.Bacc("TRN2", target_bir_lowering=False, debug=False)
    f32 = dt.float32

    packed_d = nc.dram_tensor("packed", [128, NPK], f32, kind="ExternalInput")
    # layout constants
    nodes_d = nc.dram_tensor("nodes", [G], f32, kind="ExternalInput")
    pinvT_d = nc.dram_tensor("pinvT", [G, NC_], f32, kind="ExternalInput")
    seld_d = nc.dram_tensor("seld", [N_PHASES, N_PAIRS], f32, kind="ExternalInput")
    sels_d = nc.dram_tensor("sels", [N_PHASES, N_PAIRS], f32, kind="ExternalInput")
    pinvP_d = nc.dram_tensor("pinvP", [NC_, G], f32, kind="ExternalInput")
    id6_d = nc.dram_tensor("ident6", [N_PAIRS, N_PAIRS], f32, kind="ExternalInput")
    onesr_d = nc.dram_tensor("ones_row", [1, 128], f32, kind="ExternalInput")
    onesc_d = nc.dram_tensor("ones_col", [128, 1], f32, kind="ExternalInput")
    out_d = nc.dram_tensor("out", [1, 1], f32, kind="ExternalOutput")
    if debug_outs:
        dbgc_d = nc.dram_tensor("dbg_c", [NC_, 1], f32, kind="ExternalOutput")
        dbgf_d = nc.dram_tensor("dbg_fcol", [G, 1], f32, kind="ExternalOutput")
        dbgp_d = nc.dram_tensor("dbg_phi", [NC_, 1], f32, kind="ExternalOutput")

    with tile.TileContext(nc) as tc, ExitStack() as ctx:
        cpool = ctx.enter_context(tc.tile_pool(name="consts", bufs=1))
        wpool = ctx.enter_context(tc.tile_pool(name="work", bufs=1))
        gpool = ctx.enter_context(tc.tile_pool(name="gwork", bufs=3))
        pps = ctx.enter_context(tc.tile_pool(name="pps", bufs=1, space="PSUM"))

        # ---- constants loaded once ----
        onesr_t = cpool.tile([1, 128], f32, tag="onesr")
        nc.sync.dma_start(onesr_t[:], onesr_d.ap())
        onesc_t = cpool.tile([128, 1], f32, tag="onesc")
        nc.sync.dma_start(onesc_t[:], onesc_d.ap())
        seld_t = cpool.tile([N_PHASES, N_PAIRS], f32, tag="seld")
        nc.sync.dma_start(seld_t[:], seld_d.ap())
        sels_t = cpool.tile([N_PHASES, N_PAIRS], f32, tag="sels")
        nc.sync.dma_start(sels_t[:], sels_d.ap())
        pinvP_t = cpool.tile([NC_, G], f32, tag="pinvP")
        nc.sync.dma_start(pinvP_t[:], pinvP_d.ap())
        id6_t = cpool.tile([N_PAIRS, N_PAIRS], f32, tag="id6")
        nc.sync.dma_start(id6_t[:], id6_d.ap())
        pinvT_t = cpool.tile([G, NC_], f32, tag="pinvT")
        nc.sync.dma_start(pinvT_t[:], pinvT_d.ap())
        # node coordinates replicated to all 128 partitions: [128, G]
        xrep_t = cpool.tile([128, G], f32, tag="xrep")
        nc.sync.dma_start(
            xrep_t[:],
            nodes_d.ap().rearrange("(a b) -> a b", a=1).to_broadcast((128, G)),
        )

        def body():
            # ---- packed input: params DMA first (gates ACT chain), u second
            pk_t = wpool.tile([128, NPK], f32, tag="packed")
            nc.sync.dma_start(pk_t[:, 0:C_U], packed_d.ap()[:, 0:C_U])
            nc.sync.dma_start(pk_t[:, C_U:NPK], packed_d.ap()[:, C_U:NPK])
            sncol = pk_t[:, C_SN:C_SN + 1]
            dcol6 = pk_t[0:N_PAIRS, C_D:C_D + 1]
            i4col = pk_t[:, C_I4:C_I4 + 1]
            i4_t = pk_t[0:N_PHASES, C_I4:C_I4 + 1]
            wrow = pk_t[0:1, C_W:C_W + NW]
            eps_t = pk_t[0:N_PAIRS, C_EPS:C_EPS + N_MC]
            usb = pk_t[:, C_U:C_U + SW]

            # PSUM scratch (column-sliced; 4 banks total)
            ptiny = pps.tile([128, 8], f32, tag="ptiny")
            pwide = pps.tile([128, 16], f32, tag="pwide")

            # ====== latency-critical prep that gates the ACT stream ======
            with tc.high_priority():
                # dummy Erf: forces the sigmoid-set table load to start at
                # iteration t=0 (no data deps) instead of after e1's inputs
                pre = wpool.tile([1, 1], f32, tag="pre")
                nc.scalar.activation(pre[:], onesr_t[0:1, 0:1], AF.Erf)
                # softmax prep -> sigmoid
                m11 = wpool.tile([1, 1], f32, tag="m11")
                nc.vector.reduce_max(m11[:], wrow, axis=mybir.AxisListType.X)
                wm = wpool.tile([1, NW], f32, tag="wm")
                nc.vector.tensor_scalar(wm[:], wrow, m11[:], None, ALU.subtract)
                # erf scale/bias from the replicated d column
                scale_erf = wpool.tile([N_PAIRS, 1], f32, tag="scale_erf")
                nc.vector.tensor_scalar_mul(scale_erf[:], dcol6, SQRT2)
                bias_erf = wpool.tile([N_PAIRS, 1], f32, tag="bias_erf")
                nc.vector.tensor_scalar_mul(bias_erf[:], dcol6, -1.0 / SQRT2)
                # kcol = 1/(sqrt2*sn)
                iscol = wpool.tile([128, 1], f32, tag="iscol")
                nc.vector.reciprocal(iscol[:], sncol)
                kcol = wpool.tile([128, 1], f32, tag="kcol")
                nc.vector.tensor_scalar_mul(kcol[:], iscol[:], 1.0 / SQRT2)

                # ACT: erf for interface centers (loads sigmoid set), sigmoid
                e1 = wpool.tile([N_PAIRS, N_MC], f32, tag="e1")
                nc.scalar.activation(e1[:], eps_t, AF.Erf,
                                     bias=bias_erf[:], scale=scale_erf[:])
                sig = wpool.tile([1, NW], f32, tag="sig")
                nc.scalar.activation(sig[:], wm[:], AF.Sigmoid)

                # interface centers cin [6,128] -> ccT -> biasz
                hdiff_p = ptiny[0:N_PAIRS, 0:1]
                nc.tensor.matmul(hdiff_p, seld_t[:], i4_t, start=True, stop=True)
                bsum_p = ptiny[0:N_PAIRS, 1:2]
                nc.tensor.matmul(bsum_p, sels_t[:], i4_t, start=True, stop=True)
                hdiff = wpool.tile([N_PAIRS, 1], f32, tag="hdiff")
                nc.vector.tensor_copy(hdiff[:], hdiff_p)
                cin = wpool.tile([N_PAIRS, N_MC], f32, tag="cin")
                nc.vector.tensor_scalar(cin[:], e1[:], hdiff[:], bsum_p,
                                        ALU.mult, ALU.add)
                ccT = pwide[:, 0:N_PAIRS]
                nc.tensor.transpose(ccT, cin[:], id6_t[:])
                biasz = wpool.tile([128, N_GROUPS], f32, tag="biasz")
                nc.vector.tensor_scalar(biasz[:, 0:N_PAIRS], ccT, kcol[:],
                                        -1.0, ALU.mult, ALU.mult)
                nc.vector.tensor_scalar(biasz[:, N_PAIRS:N_GROUPS], i4col,
                                        kcol[:], -1.0, ALU.mult, ALU.mult)

                # normalized weights: eec = (sig/(1-sig)) / se  (= softmax)
                omse = wpool.tile([1, NW], f32, tag="omse")
                nc.vector.tensor_scalar(omse[:], sig[:], -1.0, 1.0,
                                        ALU.mult, ALU.add)
                rec = wpool.tile([1, NW], f32, tag="rec")
                nc.vector.reciprocal(rec[:], omse[:])
                ee = wpool.tile([1, NW], f32, tag="ee")
                nc.vector.tensor_tensor(ee[:], sig[:], rec[:], ALU.mult)
                se = wpool.tile([1, 1], f32, tag="se")
                nc.vector.reduce_sum(se[:], ee[:], axis=mybir.AxisListType.X)
                rse = wpool.tile([1, 1], f32, tag="rse")
                nc.vector.reciprocal(rse[:], se[:])
                eec = wpool.tile([1, NW], f32, tag="eec")
                nc.vector.tensor_scalar(eec[:], ee[:], rse[:], None, ALU.mult)

                # weight columns [128, 7]
                p_eeT = ptiny[0:NW, 2:3]
                nc.tensor.matmul(p_eeT, eec[:], onesr_t[0:1, 0:1],
                                 start=True, stop=True)
                p_wif = pwide[:, 8:8 + N_PAIRS]
                nc.tensor.matmul(p_wif, onesr_t[:], eec[0:1, N_PHASES:],
                                 start=True, stop=True)
                wcols = wpool.tile([128, N_GROUPS], f32, tag="wcols")
                nc.vector.tensor_scalar_mul(wcols[:, 0:N_PAIRS], p_wif,
                                            1.0 / N_MC)
                nc.vector.memset(wcols[:, N_PAIRS:N_GROUPS], 0.0)
                nc.vector.tensor_copy(wcols[0:N_PHASES, N_PAIRS:N_GROUPS],
                                      p_eeT[0:N_PHASES, :])

            # ================= stream A: table + fit =================
            fcol = wpool.tile([G, 1], f32, tag="fcol")
            if 'notable' in ablate:
                nc.vector.memset(fcol[:], -1.0)
            else:
                pScol = ptiny[0:G, 3:4]
                for g in range(N_GROUPS):
                    eg = gpool.tile([128, G], f32, tag="eg")
                    nc.scalar.activation(eg[:], xrep_t[:], AF.Derivative_Erf,
                                         bias=biasz[:, g:g + 1], scale=kcol[:])
                    nc.tensor.matmul(pScol, eg[:], wcols[:, g:g + 1],
                                     start=(g == 0), stop=(g == N_GROUPS - 1))
                nc.scalar.activation(fcol[:], pScol, AF.Ln)

            # ================= stream B: moments =================
            # powers of t with fused free-dim accumulation: odd/low powers on
            # DVE (tensor_tensor_reduce), some even powers on ACT (Square
            # with accum_out).  sums[:, k] = per-partition sum of t^k.
            pows = wpool.tile([128, NC_ * SW], f32, tag="pows")  # slot d: t^d

            def slot(d):
                return pows[:, d * SW:(d + 1) * SW]

            sums = wpool.tile([128, NC_], f32, tag="sums")
            if 'nomom' in ablate:
                nc.vector.memset(sums[:], 1.0)
                nc.vector.tensor_scalar_add(sums[0:1, 0:1], usb[0:1, 0:1], 0.0)
            else:
                nc.vector.memset(sums[:, 0:1], float(SW))
                tmap = wpool.tile([128, SW], f32, tag="tmap")
                nc.vector.tensor_scalar(tmap[:], usb, MID, INV,
                                        ALU.subtract, ALU.mult)
                nc.vector.tensor_scalar(slot(1), tmap[:], -1.0, 1.0,
                                        ALU.max, ALU.min)
                nc.vector.reduce_sum(sums[:, 1:2], slot(1),
                                     axis=mybir.AxisListType.X)

                def dve_pow(k, a, b):
                    nc.vector.scalar_tensor_tensor(
                        slot(k), slot(a), 1.0, slot(b),
                        ALU.mult, ALU.mult, accum_out=sums[:, k:k + 1])

                def act_pow(k, a):
                    nc.scalar.activation(slot(k), slot(a), AF.Square,
                                         accum_out=sums[:, k:k + 1])

                dve_pow(2, 1, 1)
                dve_pow(3, 1, 2)
                dve_pow(4, 2, 2)
                dve_pow(5, 2, 3)
                dve_pow(6, 3, 3)
                dve_pow(7, 3, 4)
                act_pow(8, 4)
                dve_pow(9, 4, 5)
                act_pow(10, 5)
                dve_pow(11, 5, 6)
                act_pow(12, 6)
                dve_pow(13, 6, 7)
            phi_p = ptiny[0:NC_, 5:6]
            nc.tensor.matmul(phi_p, sums[:], onesc_t[:], start=True, stop=True)
            phi_sb = wpool.tile([NC_, 1], f32, tag="phi_sb")
            nc.vector.tensor_copy(phi_sb[:], phi_p)
            # q = pinv^T . phi (independent of the table -> ready early);
            # then sum_m P(t_m) = q . fcol needs just one matmul after the Ln
            q_p = pwide[:, 6:7]
            nc.tensor.matmul(q_p[0:G, :], pinvP_t[:], phi_sb[:],
                             start=True, stop=True)
            q_sb = wpool.tile([G, 1], f32, tag="q_sb")
            nc.vector.tensor_copy(q_sb[:], q_p[0:G, :])

            # ================= converge =================
            pout = pwide[0:1, 14:15]
            nc.tensor.matmul(pout, q_sb[:], fcol[:], start=True, stop=True)
            out_sb = wpool.tile([1, 1], f32, tag="out_sb")
            nc.vector.tensor_copy(out_sb[:], pout)
            nc.sync.dma_start(out_d.ap(), out_sb[:])
            if debug_outs:
                c_p = ptiny[0:NC_, 4:5]
                nc.tensor.matmul(c_p, pinvT_t[:], fcol[:], start=True, stop=True)
                c_sb = wpool.tile([NC_, 1], f32, tag="c_sb")
                nc.vector.tensor_copy(c_sb[:], c_p)
                nc.sync.dma_start(dbgc_d.ap(), c_sb[:])
                nc.sync.dma_start(dbgf_d.ap(), fcol[:])
                nc.sync.dma_start(dbgp_d.ap(), phi_sb[:])

        if repeat == 1:
            body()
        else:
            with tc.For_i(0, repeat, 1):
                body()

    nc.compile()
    return nc


def _consts():
    ia = np.zeros((N_PHASES, N_PAIRS), np.float32)
    ib = np.zeros((N_PHASES, N_PAIRS), np.float32)
    for p, (a, b) in enumerate(zip(_IA, _IB)):
        ia[a, p] = 1.0
        ib[b, p] = 1.0
    # Chebyshev nodes on [LO, HI] and monomial-basis fit pseudo-inverse
    i = np.arange(G)
    tnodes = np.cos(np.pi * (2 * i + 1) / (2 * G))
    xnodes = (tnodes + 1) / 2 * (HI - LO) + LO
    V = np.vander(tnodes, NC_, increasing=True)      # [G, NC_] float64
    pinvT = np.linalg.pinv(V).T.astype(np.float32)   # [G, NC_]
    return {
        "nodes": xnodes.astype(np.float32),
        "pinvT": pinvT,
        "seld": (ib - ia) * 0.5,
        "sels": (ia + ib) * 0.5,
        "pinvP": np.linalg.pinv(V).astype(np.float32),
        "ident6": np.eye(N_PAIRS, dtype=np.float32),
        "ones_row": np.ones((1, 128), np.float32),
        "ones_col": np.ones((128, 1), np.float32),
    }


def make_in_maps(u, uniform_eps, I, sigma_n, d, W):
    u = np.asarray(u, np.float32).reshape(M_TOTAL)
    sn_v = np.float32(np.asarray(sigma_n).reshape(-1)[0])
    d_v = np.float32(np.asarray(d).reshape(-1)[0])
    base = np.zeros((128, NPK), np.float32)
    base[:, C_SN] = sn_v
    base[:, C_D] = d_v
    base[0:N_PHASES, C_I4] = np.asarray(I, np.float32).reshape(N_PHASES)
    base[0, C_W:C_W + NW] = np.asarray(W, np.float32).reshape(NW)
    base[0:N_PAIRS, C_EPS:C_EPS + N_MC] = np.asarray(
        uniform_eps, np.float32).reshape(N_PAIRS, N_MC)
    consts = _consts()
    in_maps = []
    for c in range(N_CORES):
        m = dict(consts)
        pk = base.copy()
        pk[:, C_U:C_U + SW] = u[c * M_SHARD:(c + 1) * M_SHARD].reshape(128, SW)
        m["packed"] = pk
        in_maps.append(m)
    return in_maps


def kernel(u, uniform_eps, I, sigma_b, sigma_n, d, W, n_MC_components=None):
    global last_exec_time_ns, last_results
    in_maps = make_in_maps(u, uniform_eps, I, sigma_n, d, W)

    key = "nc_dbg" if os.environ.get("KERNEL_DEBUG") else "nc"
    if key not in _cache:
        _cache[key] = _build_nc(debug_outs=bool(os.environ.get("KERNEL_DEBUG")))
    nc = _cache[key]

    trace = bool(int(os.environ.get("KERNEL_TRACE", "0")))
    res = run_bass_kernel_spmd(nc, in_maps, core_ids=list(range(N_CORES)),
                               trace=trace)
    last_results = res
    last_exec_time_ns = res.exec_time_ns

    total = sum(float(res.results[c]["out"][0, 0]) for c in range(N_CORES))
    sn_f = float(np.asarray(sigma_n).reshape(-1)[0])
    c0 = math.log(math.sqrt(math.pi) / 2.0) - math.log(math.sqrt(2.0 * math.pi) * sn_f)
    loss = -(total / M_TOTAL + c0)
    return np.float32(loss)
